# revision 18
# baseline (speedup 1.0000x reference)
"""Stride-2 bilinear upsampling (block-diagonal conv_transpose2d) on 8 NeuronCores.

v4 design: bf16 end-to-end on device, and fully COLUMN-PLANAR on device.

The device never materializes interleaved output columns: it computes the
even-column plane and odd-column plane separately and writes two bf16 HBM
tensors; the host interleaves them (pure layout, part of the unshard) and
upcasts to f32.  The a^2 = 1/16 filter scale is folded into the host-side
bf16 cast (power of two -> lossless).  This removes every strided-dst DVE
op, so all tensor_tensor ops hit the 2x bf16 uop (~0.59 ns/elem measured)
and all tensor_scalar ops can be flat-contiguous for the 4x uop
(~0.28 ns/elem; 2D access patterns demote ts to 1x, measured).

Math (per 1D axis, taps f = [1,3,3,1]*a): with S = a^2*X (host), ghosts 0:
  W-pass (cols, planar):  Ue[u,j] = 3S[j+1] + S[j+2] = T[j+1] + S[j+2]
                          Uo[u,j] = S[j+1] + 3S[j+2] = S[j+1] + T[j+2]
                          (T = 3S via flat ts 4x)
  H-pass (rows, per plane P in {e,o}):  Zp[2m]   = 3Up[m] + Up[m+1]
                                        Zp[2m+1] = Up[m] + 3Up[m+1]
                          (Vp = 3Up on ACT; row-interleaved dst is packed
                           in the inner dim so tt stays 2x)
Engine split: DVE = T + W + most H; ACT = V (+T on non-edge strips);
Pool (GpSimd tt, slow ~3.8 ns/elem) absorbs the odd plane of every k-th
strip as a self-contained chain (tt, tt, own SWDGE DMA).

Channel-parallel: 32 ch x 4 batch = 128 images/core, one per partition.
"""

import numpy as np

N, C, H, W = 4, 256, 128, 128
OH, OW = 258, 258
PW = OW // 2 + 1           # plane width padded to 130 (even => 2x uop, 4B-aligned rows)
NCORES = 8
CPC = C // NCORES          # 32 channels per core
NIMG = N * CPC             # 128 images per core (one per SBUF partition)
SW = W + 4                 # S width: [pad][ghostL][128 data][ghostR][pad]

_CACHE = {}


def _legalize_waits(nc, mybir):
    """Split multi-wait sync_info into standalone single-wait EventSemaphore
    instructions (this build encodes at most one sync-wait per instruction)."""
    n = 0
    for func in nc.m.functions:
        for block in func.blocks:
            out = []
            for inst in block.instructions:
                si = inst.sync_info
                if si is not None and si.on_wait is not None and len(si.on_wait) > 1:
                    waits = list(si.on_wait)
                    for k, w in enumerate(waits[:-1]):
                        out.append(mybir.InstEventSemaphore(
                            name=f"{inst.name}-hw{k}",
                            opcode="EventSemaphore",
                            engine=inst.engine,
                            ins=[], outs=[],
                            sync_info=mybir.SyncInfo(on_wait=[w], on_update=[]),
                        ))
                        n += 1
                    inst.sync_info = mybir.SyncInfo(
                        on_wait=[waits[-1]], on_update=list(si.on_update))
                out.append(inst)
            block.instructions = out
    return n


def _build_bass(strips=None, bufs_v=3, bufs_z=4, bufs_t=3,
                pool_every=3, t_act=1, v_act=1, v_dve_edge=4,
                out_scalar_every=0, out_delay=2):
    """Per-core view: x[128,128,128]bf16 (pre-scaled by a^2) ->
    out_e/out_o[128,258,129]bf16 (even/odd column planes).

    pool_every: every k-th non-edge strip's odd plane H-pass runs on
    GpSimd (0 = never).  t_act/v_act: put T=3S / V=3U on ACT for non-edge
    strips (else DVE flat ts 4x).  v_dve_edge: first/last k strips keep
    everything on DVE for short fill/drain latency.
    """
    import concourse.bass as bass
    import concourse.mybir as mybir
    from concourse.tile import TileContext

    bf16 = mybir.dt.bfloat16
    Copy = mybir.ActivationFunctionType.Copy
    add = mybir.AluOpType.add
    if strips is None:
        strips = [1, 1, 2, 4] + [8] * 14 + [4, 2, 1, 1]
    assert sum(strips) == H
    nstrips = len(strips)
    hs_max = max(strips)
    m0s = list(np.cumsum([0] + strips[:-1]))

    nc = bass.Bass()
    x = nc.dram_tensor("x", [NIMG, H, W], bf16, kind="ExternalInput")
    out_e = nc.dram_tensor("out_e", [NIMG, OH, PW], bf16,
                           kind="ExternalOutput")
    out_o = nc.dram_tensor("out_o", [NIMG, OH, PW], bf16,
                           kind="ExternalOutput")

    with TileContext(nc) as tc:
        with tc.tile_pool(name="p", bufs=2) as pool:
            # persistent S: input rows land at cols 2..129; ghost zeros at
            # cols 1/130; pad cols 0/131 also zeroed (T reads full rows so
            # its flat access pattern stays contiguous).
            S = pool.tile([NIMG, H, SW], bf16, tag="S", bufs=1)
            nc.gpsimd.memset(S[:, :, 0:2], 0.0)
            nc.gpsimd.memset(S[:, :, SW - 2:SW], 0.0)

            # persistent planar U [130 x 129] per parity; ghost rows 0/129.
            Ue = pool.tile([NIMG, H + 2, PW], bf16, tag="Ue", bufs=1)
            Uo = pool.tile([NIMG, H + 2, PW], bf16, tag="Uo", bufs=1)
            for Up in (Ue, Uo):
                nc.gpsimd.memset(Up[:, 0:1, :], 0.0)
                nc.gpsimd.memset(Up[:, H + 1:H + 2, :], 0.0)

            # input chunks aligned to strip boundaries, scalar HWDGE ring
            in_chunks = [(0, 1), (1, 1), (2, 2), (4, 4), (8, 24), (32, 24),
                         (56, 24), (80, 24), (104, 24)]
            assert sum(n for _, n in in_chunks) == H
            for ci, (r0, nr) in enumerate(in_chunks):
                e = nc.sync if nr <= 4 else nc.gpsimd
                e.dma_start(out=S[:, r0:r0 + nr, 2:2 + W],
                            in_=x[:, r0:r0 + nr, :])

            pending = []   # (due_strip, zt_view, dram, row0, row1)

            def flush_pending(now):
                while pending and pending[0][0] <= now:
                    _, ztv, dram, r0_, r1_ = pending.pop(0)
                    nc.scalar.dma_start(out=dram[:, r0_:r1_, :], in_=ztv)

            npool = 0
            for s in range(nstrips):
                hs = strips[s]
                m0 = int(m0s[s])
                n_m = hs + (1 if s == nstrips - 1 else 0)
                edge = s < v_dve_edge or s >= nstrips - v_dve_edge

                # T = 3S over full S rows (flat on DVE for 4x; ACT is
                # stride-blind). T local col c corresponds to S col c.
                tt_ = pool.tile([NIMG, hs_max, SW], bf16, tag="tt",
                                bufs=bufs_t)
                if t_act and not edge:
                    nc.scalar.activation(tt_[:, 0:hs, :], S[:, m0:m0 + hs, :],
                                         Copy, scale=3.0)
                else:
                    nc.vector.tensor_scalar_mul(
                        tt_[:, 0:hs, :], S[:, m0:m0 + hs, :], 3.0)

                # W-pass (planar, packed tt 2x) -> U rows m0+1 .. m0+hs
                nc.vector.tensor_tensor(
                    out=Ue[:, m0 + 1:m0 + hs + 1, :],
                    in0=tt_[:, 0:hs, 1:W + 3],
                    in1=S[:, m0:m0 + hs, 2:W + 4], op=add)
                nc.vector.tensor_tensor(
                    out=Uo[:, m0 + 1:m0 + hs + 1, :],
                    in0=S[:, m0:m0 + hs, 1:W + 3],
                    in1=tt_[:, 0:hs, 2:W + 4], op=add)

                # whole odd plane of every k-th non-edge strip -> Pool
                pool_o = (pool_every and not edge
                          and (s - v_dve_edge) % pool_every == pool_every - 1)
                if pool_o:
                    npool += 1

                for par, Up, dram in ((0, Ue, out_e), (1, Uo, out_o)):
                    on_pool = pool_o and par == 1
                    # V = 3U rows m0 .. m0+n_m (flat ts 4x on DVE, or ACT)
                    vt_full = pool.tile([NIMG, hs_max + 2, PW], bf16,
                                        tag=f"vt{par}", bufs=bufs_v)
                    vt = vt_full[:, 0:n_m + 1, :]
                    if v_act and not edge:
                        nc.scalar.activation(vt, U_rows(Up, m0, n_m),
                                             Copy, scale=3.0)
                    else:
                        nc.vector.tensor_scalar_mul(
                            vt, U_rows(Up, m0, n_m), 3.0)

                    # H-pass: Zp rows 2m0 .. 2(m0+n_m)-1
                    he = nc.gpsimd if on_pool else nc.vector
                    zt_full = pool.tile([NIMG, 2 * hs_max + 2, PW], bf16,
                                        tag=f"zt{par}", bufs=bufs_z)
                    zt = zt_full[:, 0:2 * n_m, :]
                    he.tensor_tensor(
                        out=zt[:, 0:2 * n_m:2, :],
                        in0=vt[:, 0:n_m, :],
                        in1=Up[:, m0 + 1:m0 + n_m + 1, :], op=add)
                    he.tensor_tensor(
                        out=zt[:, 1:2 * n_m:2, :],
                        in0=Up[:, m0:m0 + n_m, :],
                        in1=vt[:, 1:n_m + 1, :], op=add)

                    r0_, r1_ = 2 * m0, 2 * (m0 + n_m)
                    if out_scalar_every and s % out_scalar_every == 1:
                        pending.append((s + out_delay, zt[:, :, :],
                                        dram, r0_, r1_))
                    else:
                        oeng = nc.gpsimd if par == 1 else nc.sync
                        oeng.dma_start(out=dram[:, r0_:r1_, :],
                                       in_=zt[:, :, :])
                flush_pending(s)
            flush_pending(nstrips)

    _legalize_waits(nc, mybir)
    return nc


def U_rows(Up, m0, n_m):
    return Up[:, m0:m0 + n_m + 1, :]


def _taps_from_w(w):
    """Recover separable 4-tap filter f (filt = outer(f, f)) from w[0, 0];
    return a^2 where f = [a, 3a, 3a, a]."""
    filt = np.asarray(w, dtype=np.float32)[0, 0]
    j = int(np.argmax(np.abs(np.diag(filt))))
    f = filt[:, j] / np.float32(np.sqrt(filt[j, j]))
    assert np.allclose(np.outer(f, f), filt, atol=1e-5), "filter not separable"
    assert abs(f[0] - f[3]) < 1e-6 and abs(f[1] - f[2]) < 1e-6, "not symmetric"
    assert abs(f[1] - 3 * f[0]) < 1e-5, "not the 3:1 bilinear tap"
    return float(f[0]) * float(f[0])


BEST_CFG = dict(strips=[2, 6] + [16] * 7 + [6, 2],
                pool_every=0, v_dve_edge=2, bufs_z=2, bufs_v=3)


def _get_nc(**cfg):
    cfg = {**BEST_CFG, **cfg}
    key = tuple(sorted(
        (k, tuple(v) if isinstance(v, list) else v) for k, v in cfg.items()))
    if key not in _CACHE:
        _CACHE[key] = _build_bass(**cfg)
    return _CACHE[key]


def run_sharded(x, w, cfg=None, **run_kwargs):
    import ml_dtypes
    from concourse.bass_utils import run_bass_kernel_spmd

    scale = _taps_from_w(w)
    nc = _get_nc(**(cfg or {}))

    x = np.asarray(x, dtype=np.float32)
    in_maps = []
    for k in range(NCORES):
        xk = (x[:, k * CPC:(k + 1) * CPC].reshape(NIMG, H, W)
              * np.float32(scale)).astype(ml_dtypes.bfloat16)
        in_maps.append({"x": np.ascontiguousarray(xk)})

    res = run_bass_kernel_spmd(nc, in_maps, core_ids=list(range(NCORES)),
                               **run_kwargs)

    full = np.empty((N, C, OH, OW), dtype=np.float32)
    for k in range(NCORES):
        e = res.results[k]["out_e"].reshape(N, CPC, OH, PW)
        o = res.results[k]["out_o"].reshape(N, CPC, OH, PW)
        # interleave column planes: out[..., 2j] = e[..., j], 2j+1 = o[..., j]
        full[:, k * CPC:(k + 1) * CPC] = np.stack(
            [e, o], axis=-1).reshape(N, CPC, OH, 2 * PW)[..., :OW].astype(
            np.float32)
    return full, res


def kernel(x, w):
    full, _ = run_sharded(x, w)
    return full


# revision 19
# speedup vs baseline: 1.0390x; 1.0390x over previous
"""Stride-2 bilinear upsampling (block-diagonal conv_transpose2d) on 8 NeuronCores.

v4 design: bf16 end-to-end on device, and fully COLUMN-PLANAR on device.

The device never materializes interleaved output columns: it computes the
even-column plane and odd-column plane separately and writes two bf16 HBM
tensors; the host interleaves them (pure layout, part of the unshard) and
upcasts to f32.  The a^2 = 1/16 filter scale is folded into the host-side
bf16 cast (power of two -> lossless).  This removes every strided-dst DVE
op, so all tensor_tensor ops hit the 2x bf16 uop (~0.59 ns/elem measured)
and all tensor_scalar ops can be flat-contiguous for the 4x uop
(~0.28 ns/elem; 2D access patterns demote ts to 1x, measured).

Math (per 1D axis, taps f = [1,3,3,1]*a): with S = a^2*X (host), ghosts 0:
  W-pass (cols, planar):  Ue[u,j] = 3S[j+1] + S[j+2] = T[j+1] + S[j+2]
                          Uo[u,j] = S[j+1] + 3S[j+2] = S[j+1] + T[j+2]
                          (T = 3S via flat ts 4x)
  H-pass (rows, per plane P in {e,o}):  Zp[2m]   = 3Up[m] + Up[m+1]
                                        Zp[2m+1] = Up[m] + 3Up[m+1]
                          (Vp = 3Up on ACT; row-interleaved dst is packed
                           in the inner dim so tt stays 2x)
Engine split: DVE = T + W + most H; ACT = V (+T on non-edge strips);
Pool (GpSimd tt, slow ~3.8 ns/elem) absorbs the odd plane of every k-th
strip as a self-contained chain (tt, tt, own SWDGE DMA).

Channel-parallel: 32 ch x 4 batch = 128 images/core, one per partition.
"""

import numpy as np

N, C, H, W = 4, 256, 128, 128
OH, OW = 258, 258
PW = OW // 2 + 1           # plane width padded to 130 (even => 2x uop, 4B-aligned rows)
NCORES = 8
CPC = C // NCORES          # 32 channels per core
NIMG = N * CPC             # 128 images per core (one per SBUF partition)
SW = W + 4                 # S width: [pad][ghostL][128 data][ghostR][pad]

_CACHE = {}


def _legalize_waits(nc, mybir):
    """Split multi-wait sync_info into standalone single-wait EventSemaphore
    instructions (this build encodes at most one sync-wait per instruction)."""
    n = 0
    for func in nc.m.functions:
        for block in func.blocks:
            out = []
            for inst in block.instructions:
                si = inst.sync_info
                if si is not None and si.on_wait is not None and len(si.on_wait) > 1:
                    waits = list(si.on_wait)
                    for k, w in enumerate(waits[:-1]):
                        out.append(mybir.InstEventSemaphore(
                            name=f"{inst.name}-hw{k}",
                            opcode="EventSemaphore",
                            engine=inst.engine,
                            ins=[], outs=[],
                            sync_info=mybir.SyncInfo(on_wait=[w], on_update=[]),
                        ))
                        n += 1
                    inst.sync_info = mybir.SyncInfo(
                        on_wait=[waits[-1]], on_update=list(si.on_update))
                out.append(inst)
            block.instructions = out
    return n


def _build_bass(strips=None, bufs_v=3, bufs_z=4, bufs_t=3,
                pool_every=3, t_act=1, v_act=1, v_dve_edge=4,
                out_scalar_every=0, out_delay=2):
    """Per-core view: x[128,128,128]bf16 (pre-scaled by a^2) ->
    out_e/out_o[128,258,129]bf16 (even/odd column planes).

    pool_every: every k-th non-edge strip's odd plane H-pass runs on
    GpSimd (0 = never).  t_act/v_act: put T=3S / V=3U on ACT for non-edge
    strips (else DVE flat ts 4x).  v_dve_edge: first/last k strips keep
    everything on DVE for short fill/drain latency.
    """
    import concourse.bass as bass
    import concourse.mybir as mybir
    from concourse.tile import TileContext

    bf16 = mybir.dt.bfloat16
    Copy = mybir.ActivationFunctionType.Copy
    add = mybir.AluOpType.add
    if strips is None:
        strips = [1, 1, 2, 4] + [8] * 14 + [4, 2, 1, 1]
    assert sum(strips) == H
    nstrips = len(strips)
    hs_max = max(strips)
    m0s = list(np.cumsum([0] + strips[:-1]))

    nc = bass.Bass()
    x = nc.dram_tensor("x", [NIMG, H, W], bf16, kind="ExternalInput")
    out_e = nc.dram_tensor("out_e", [NIMG, OH, PW], bf16,
                           kind="ExternalOutput")
    out_o = nc.dram_tensor("out_o", [NIMG, OH, PW], bf16,
                           kind="ExternalOutput")

    with TileContext(nc) as tc:
        with tc.tile_pool(name="p", bufs=2) as pool:
            # persistent S: input rows land at cols 2..129; ghost zeros at
            # cols 1/130; pad cols 0/131 also zeroed (T reads full rows so
            # its flat access pattern stays contiguous).
            S = pool.tile([NIMG, H, SW], bf16, tag="S", bufs=1)
            nc.gpsimd.memset(S[:, :, 0:2], 0.0)
            nc.gpsimd.memset(S[:, :, SW - 2:SW], 0.0)

            # persistent planar U [130 x 129] per parity; ghost rows 0/129.
            Ue = pool.tile([NIMG, H + 2, PW], bf16, tag="Ue", bufs=1)
            Uo = pool.tile([NIMG, H + 2, PW], bf16, tag="Uo", bufs=1)
            for Up in (Ue, Uo):
                nc.gpsimd.memset(Up[:, 0:1, :], 0.0)
                nc.gpsimd.memset(Up[:, H + 1:H + 2, :], 0.0)

            # input chunks aligned to strip boundaries, scalar HWDGE ring
            in_chunks = [(0, 1), (1, 1), (2, 2), (4, 4), (8, 24), (32, 24),
                         (56, 24), (80, 24), (104, 24)]
            assert sum(n for _, n in in_chunks) == H
            for ci, (r0, nr) in enumerate(in_chunks):
                e = nc.sync if nr <= 4 else nc.gpsimd
                e.dma_start(out=S[:, r0:r0 + nr, 2:2 + W],
                            in_=x[:, r0:r0 + nr, :])

            pending = []   # (due_strip, zt_view, dram, row0, row1)

            def flush_pending(now):
                while pending and pending[0][0] <= now:
                    _, ztv, dram, r0_, r1_ = pending.pop(0)
                    nc.scalar.dma_start(out=dram[:, r0_:r1_, :], in_=ztv)

            npool = 0
            for s in range(nstrips):
                hs = strips[s]
                m0 = int(m0s[s])
                n_m = hs + (1 if s == nstrips - 1 else 0)
                edge = s < v_dve_edge or s >= nstrips - v_dve_edge

                # T = 3S over full S rows (flat on DVE for 4x; ACT is
                # stride-blind). T local col c corresponds to S col c.
                tt_ = pool.tile([NIMG, hs_max, SW], bf16, tag="tt",
                                bufs=bufs_t)
                if t_act and not edge:
                    nc.scalar.activation(tt_[:, 0:hs, :], S[:, m0:m0 + hs, :],
                                         Copy, scale=3.0)
                else:
                    nc.vector.tensor_scalar_mul(
                        tt_[:, 0:hs, :], S[:, m0:m0 + hs, :], 3.0)

                # W-pass (planar, packed tt 2x) -> U rows m0+1 .. m0+hs
                nc.vector.tensor_tensor(
                    out=Ue[:, m0 + 1:m0 + hs + 1, :],
                    in0=tt_[:, 0:hs, 1:W + 3],
                    in1=S[:, m0:m0 + hs, 2:W + 4], op=add)
                nc.vector.tensor_tensor(
                    out=Uo[:, m0 + 1:m0 + hs + 1, :],
                    in0=S[:, m0:m0 + hs, 1:W + 3],
                    in1=tt_[:, 0:hs, 2:W + 4], op=add)

                # whole odd plane of every k-th non-edge strip -> Pool
                pool_o = (pool_every and not edge
                          and (s - v_dve_edge) % pool_every == pool_every - 1)
                if pool_o:
                    npool += 1

                for par, Up, dram in ((0, Ue, out_e), (1, Uo, out_o)):
                    on_pool = pool_o and par == 1
                    # V = 3U rows m0 .. m0+n_m (flat ts 4x on DVE, or ACT)
                    vt_full = pool.tile([NIMG, hs_max + 2, PW], bf16,
                                        tag=f"vt{par}", bufs=bufs_v)
                    vt = vt_full[:, 0:n_m + 1, :]
                    if v_act and not edge:
                        nc.scalar.activation(vt, U_rows(Up, m0, n_m),
                                             Copy, scale=3.0)
                    else:
                        nc.vector.tensor_scalar_mul(
                            vt, U_rows(Up, m0, n_m), 3.0)

                    # H-pass: Zp rows 2m0 .. 2(m0+n_m)-1
                    he = nc.gpsimd if on_pool else nc.vector
                    zt_full = pool.tile([NIMG, 2 * hs_max + 2, PW], bf16,
                                        tag=f"zt{par}", bufs=bufs_z)
                    zt = zt_full[:, 0:2 * n_m, :]
                    he.tensor_tensor(
                        out=zt[:, 0:2 * n_m:2, :],
                        in0=vt[:, 0:n_m, :],
                        in1=Up[:, m0 + 1:m0 + n_m + 1, :], op=add)
                    he.tensor_tensor(
                        out=zt[:, 1:2 * n_m:2, :],
                        in0=Up[:, m0:m0 + n_m, :],
                        in1=vt[:, 1:n_m + 1, :], op=add)

                    r0_, r1_ = 2 * m0, 2 * (m0 + n_m)
                    if out_scalar_every and s % out_scalar_every == 1:
                        pending.append((s + out_delay, zt[:, :, :],
                                        dram, r0_, r1_))
                    else:
                        oeng = nc.gpsimd if par == 1 else nc.sync
                        oeng.dma_start(out=dram[:, r0_:r1_, :],
                                       in_=zt[:, :, :])
                flush_pending(s)
            flush_pending(nstrips)

    _legalize_waits(nc, mybir)
    return nc


def U_rows(Up, m0, n_m):
    return Up[:, m0:m0 + n_m + 1, :]


def _taps_from_w(w):
    """Recover separable 4-tap filter f (filt = outer(f, f)) from w[0, 0];
    return a^2 where f = [a, 3a, 3a, a]."""
    filt = np.asarray(w, dtype=np.float32)[0, 0]
    j = int(np.argmax(np.abs(np.diag(filt))))
    f = filt[:, j] / np.float32(np.sqrt(filt[j, j]))
    assert np.allclose(np.outer(f, f), filt, atol=1e-5), "filter not separable"
    assert abs(f[0] - f[3]) < 1e-6 and abs(f[1] - f[2]) < 1e-6, "not symmetric"
    assert abs(f[1] - 3 * f[0]) < 1e-5, "not the 3:1 bilinear tap"
    return float(f[0]) * float(f[0])


BEST_CFG = dict(strips=[2, 6] + [24] * 4 + [16, 6, 2],
                pool_every=0, v_dve_edge=2, bufs_z=2, bufs_v=2, bufs_t=2)


def _get_nc(**cfg):
    cfg = {**BEST_CFG, **cfg}
    key = tuple(sorted(
        (k, tuple(v) if isinstance(v, list) else v) for k, v in cfg.items()))
    if key not in _CACHE:
        _CACHE[key] = _build_bass(**cfg)
    return _CACHE[key]


def run_sharded(x, w, cfg=None, **run_kwargs):
    import ml_dtypes
    from concourse.bass_utils import run_bass_kernel_spmd

    scale = _taps_from_w(w)
    nc = _get_nc(**(cfg or {}))

    x = np.asarray(x, dtype=np.float32)
    in_maps = []
    for k in range(NCORES):
        xk = (x[:, k * CPC:(k + 1) * CPC].reshape(NIMG, H, W)
              * np.float32(scale)).astype(ml_dtypes.bfloat16)
        in_maps.append({"x": np.ascontiguousarray(xk)})

    res = run_bass_kernel_spmd(nc, in_maps, core_ids=list(range(NCORES)),
                               **run_kwargs)

    full = np.empty((N, C, OH, OW), dtype=np.float32)
    for k in range(NCORES):
        e = res.results[k]["out_e"].reshape(N, CPC, OH, PW)
        o = res.results[k]["out_o"].reshape(N, CPC, OH, PW)
        # interleave column planes: out[..., 2j] = e[..., j], 2j+1 = o[..., j]
        full[:, k * CPC:(k + 1) * CPC] = np.stack(
            [e, o], axis=-1).reshape(N, CPC, OH, 2 * PW)[..., :OW].astype(
            np.float32)
    return full, res


def kernel(x, w):
    full, _ = run_sharded(x, w)
    return full


# revision 20
# speedup vs baseline: 1.0792x; 1.0387x over previous
"""Stride-2 bilinear upsampling (block-diagonal conv_transpose2d) on 8 NeuronCores.

v4 design: bf16 end-to-end on device, and fully COLUMN-PLANAR on device.

The device never materializes interleaved output columns: it computes the
even-column plane and odd-column plane separately and writes two bf16 HBM
tensors; the host interleaves them (pure layout, part of the unshard) and
upcasts to f32.  The a^2 = 1/16 filter scale is folded into the host-side
bf16 cast (power of two -> lossless).  This removes every strided-dst DVE
op, so all tensor_tensor ops hit the 2x bf16 uop (~0.59 ns/elem measured)
and all tensor_scalar ops can be flat-contiguous for the 4x uop
(~0.28 ns/elem; 2D access patterns demote ts to 1x, measured).

Math (per 1D axis, taps f = [1,3,3,1]*a): with S = a^2*X (host), ghosts 0:
  W-pass (cols, planar):  Ue[u,j] = 3S[j+1] + S[j+2] = T[j+1] + S[j+2]
                          Uo[u,j] = S[j+1] + 3S[j+2] = S[j+1] + T[j+2]
                          (T = 3S via flat ts 4x)
  H-pass (rows, per plane P in {e,o}):  Zp[2m]   = 3Up[m] + Up[m+1]
                                        Zp[2m+1] = Up[m] + 3Up[m+1]
                          (Vp = 3Up on ACT; row-interleaved dst is packed
                           in the inner dim so tt stays 2x)
Engine split: DVE = T + W + most H; ACT = V (+T on non-edge strips);
Pool (GpSimd tt, slow ~3.8 ns/elem) absorbs the odd plane of every k-th
strip as a self-contained chain (tt, tt, own SWDGE DMA).

Channel-parallel: 32 ch x 4 batch = 128 images/core, one per partition.
"""

import numpy as np

N, C, H, W = 4, 256, 128, 128
OH, OW = 258, 258
PW = OW // 2 + 1           # plane width padded to 130 (even => 2x uop, 4B-aligned rows)
NCORES = 8
CPC = C // NCORES          # 32 channels per core
NIMG = N * CPC             # 128 images per core (one per SBUF partition)
SW = W + 4                 # S width: [pad][ghostL][128 data][ghostR][pad]

_CACHE = {}


def _legalize_waits(nc, mybir):
    """Split multi-wait sync_info into standalone single-wait EventSemaphore
    instructions (this build encodes at most one sync-wait per instruction)."""
    n = 0
    for func in nc.m.functions:
        for block in func.blocks:
            out = []
            for inst in block.instructions:
                si = inst.sync_info
                if si is not None and si.on_wait is not None and len(si.on_wait) > 1:
                    waits = list(si.on_wait)
                    for k, w in enumerate(waits[:-1]):
                        out.append(mybir.InstEventSemaphore(
                            name=f"{inst.name}-hw{k}",
                            opcode="EventSemaphore",
                            engine=inst.engine,
                            ins=[], outs=[],
                            sync_info=mybir.SyncInfo(on_wait=[w], on_update=[]),
                        ))
                        n += 1
                    inst.sync_info = mybir.SyncInfo(
                        on_wait=[waits[-1]], on_update=list(si.on_update))
                out.append(inst)
            block.instructions = out
    return n


def _build_bass(strips=None, bufs_v=3, bufs_z=4, bufs_t=3,
                pool_every=3, t_act=1, v_act=1, v_dve_edge=4,
                out_scalar_every=0, out_delay=2):
    """Per-core view: x[128,128,128]bf16 (pre-scaled by a^2) ->
    out_e/out_o[128,258,129]bf16 (even/odd column planes).

    pool_every: every k-th non-edge strip's odd plane H-pass runs on
    GpSimd (0 = never).  t_act/v_act: put T=3S / V=3U on ACT for non-edge
    strips (else DVE flat ts 4x).  v_dve_edge: first/last k strips keep
    everything on DVE for short fill/drain latency.
    """
    import concourse.bass as bass
    import concourse.mybir as mybir
    from concourse.tile import TileContext

    bf16 = mybir.dt.bfloat16
    Copy = mybir.ActivationFunctionType.Copy
    add = mybir.AluOpType.add
    if strips is None:
        strips = [1, 1, 2, 4] + [8] * 14 + [4, 2, 1, 1]
    assert sum(strips) == H
    nstrips = len(strips)
    hs_max = max(strips)
    m0s = list(np.cumsum([0] + strips[:-1]))

    nc = bass.Bass()
    x = nc.dram_tensor("x", [NIMG, H, SW], bf16, kind="ExternalInput")
    out_e = nc.dram_tensor("out_e", [NIMG, OH, PW], bf16,
                           kind="ExternalOutput")
    out_o = nc.dram_tensor("out_o", [NIMG, OH, PW], bf16,
                           kind="ExternalOutput")

    with TileContext(nc) as tc:
        with tc.tile_pool(name="p", bufs=2) as pool:
            # persistent S: input rows land at cols 2..129; ghost zeros at
            # cols 1/130; pad cols 0/131 also zeroed (T reads full rows so
            # its flat access pattern stays contiguous).
            S = pool.tile([NIMG, H, SW], bf16, tag="S", bufs=1)

            # persistent planar U [130 x 129] per parity; ghost rows 0/129.
            Ue = pool.tile([NIMG, H + 2, PW], bf16, tag="Ue", bufs=1)
            Uo = pool.tile([NIMG, H + 2, PW], bf16, tag="Uo", bufs=1)
            for Up in (Ue, Uo):
                nc.gpsimd.memset(Up[:, 0:1, :], 0.0)
                nc.gpsimd.memset(Up[:, H + 1:H + 2, :], 0.0)

            # input chunks (host-padded to S layout: fully contiguous
            # 6KB-per-partition lines) aligned to strip boundaries
            in_chunks = [(0, 2), (2, 6), (8, 24), (32, 24),
                         (56, 24), (80, 24), (104, 24)]
            assert sum(n for _, n in in_chunks) == H
            for r0, nr in in_chunks:
                nc.sync.dma_start(out=S[:, r0:r0 + nr, :],
                                  in_=x[:, r0:r0 + nr, :])

            pending = []   # (due_strip, zt_view, dram, row0, row1)

            def flush_pending(now):
                while pending and pending[0][0] <= now:
                    _, ztv, dram, r0_, r1_ = pending.pop(0)
                    nc.scalar.dma_start(out=dram[:, r0_:r1_, :], in_=ztv)

            npool = 0
            for s in range(nstrips):
                hs = strips[s]
                m0 = int(m0s[s])
                n_m = hs + (1 if s == nstrips - 1 else 0)
                edge = s < v_dve_edge or s >= nstrips - v_dve_edge

                # T = 3S over full S rows (flat on DVE for 4x; ACT is
                # stride-blind). T local col c corresponds to S col c.
                tt_ = pool.tile([NIMG, hs_max, SW], bf16, tag="tt",
                                bufs=bufs_t)
                on_act = t_act and not edge and (t_act == 1 or s % 2 == 0)
                if on_act:
                    nc.scalar.activation(tt_[:, 0:hs, :], S[:, m0:m0 + hs, :],
                                         Copy, scale=3.0)
                else:
                    nc.vector.tensor_scalar_mul(
                        tt_[:, 0:hs, :], S[:, m0:m0 + hs, :], 3.0)

                # W-pass (planar, packed tt 2x) -> U rows m0+1 .. m0+hs
                nc.vector.tensor_tensor(
                    out=Ue[:, m0 + 1:m0 + hs + 1, :],
                    in0=tt_[:, 0:hs, 1:W + 3],
                    in1=S[:, m0:m0 + hs, 2:W + 4], op=add)
                nc.vector.tensor_tensor(
                    out=Uo[:, m0 + 1:m0 + hs + 1, :],
                    in0=S[:, m0:m0 + hs, 1:W + 3],
                    in1=tt_[:, 0:hs, 2:W + 4], op=add)

                # whole odd plane of every k-th non-edge strip -> Pool
                pool_o = (pool_every and not edge
                          and (s - v_dve_edge) % pool_every == pool_every - 1)
                if pool_o:
                    npool += 1

                for par, Up, dram in ((0, Ue, out_e), (1, Uo, out_o)):
                    on_pool = pool_o and par == 1
                    # V = 3U rows m0 .. m0+n_m (flat ts 4x on DVE, or ACT)
                    vt_full = pool.tile([NIMG, hs_max + 2, PW], bf16,
                                        tag=f"vt{par}", bufs=bufs_v)
                    vt = vt_full[:, 0:n_m + 1, :]
                    if v_act and not edge:
                        nc.scalar.activation(vt, U_rows(Up, m0, n_m),
                                             Copy, scale=3.0)
                    else:
                        nc.vector.tensor_scalar_mul(
                            vt, U_rows(Up, m0, n_m), 3.0)

                    # H-pass: Zp rows 2m0 .. 2(m0+n_m)-1
                    he = nc.gpsimd if on_pool else nc.vector
                    zt_full = pool.tile([NIMG, 2 * hs_max + 2, PW], bf16,
                                        tag=f"zt{par}", bufs=bufs_z)
                    zt = zt_full[:, 0:2 * n_m, :]
                    he.tensor_tensor(
                        out=zt[:, 0:2 * n_m:2, :],
                        in0=vt[:, 0:n_m, :],
                        in1=Up[:, m0 + 1:m0 + n_m + 1, :], op=add)
                    he.tensor_tensor(
                        out=zt[:, 1:2 * n_m:2, :],
                        in0=Up[:, m0:m0 + n_m, :],
                        in1=vt[:, 1:n_m + 1, :], op=add)

                    r0_, r1_ = 2 * m0, 2 * (m0 + n_m)
                    if out_scalar_every and s % out_scalar_every == 1:
                        pending.append((s + out_delay, zt[:, :, :],
                                        dram, r0_, r1_))
                    else:
                        oeng = nc.gpsimd if par == 1 else nc.sync
                        oeng.dma_start(out=dram[:, r0_:r1_, :],
                                       in_=zt[:, :, :])
                flush_pending(s)
            flush_pending(nstrips)

    _legalize_waits(nc, mybir)
    return nc


def U_rows(Up, m0, n_m):
    return Up[:, m0:m0 + n_m + 1, :]


def _taps_from_w(w):
    """Recover separable 4-tap filter f (filt = outer(f, f)) from w[0, 0];
    return a^2 where f = [a, 3a, 3a, a]."""
    filt = np.asarray(w, dtype=np.float32)[0, 0]
    j = int(np.argmax(np.abs(np.diag(filt))))
    f = filt[:, j] / np.float32(np.sqrt(filt[j, j]))
    assert np.allclose(np.outer(f, f), filt, atol=1e-5), "filter not separable"
    assert abs(f[0] - f[3]) < 1e-6 and abs(f[1] - f[2]) < 1e-6, "not symmetric"
    assert abs(f[1] - 3 * f[0]) < 1e-5, "not the 3:1 bilinear tap"
    return float(f[0]) * float(f[0])


BEST_CFG = dict(strips=[2, 6] + [24] * 4 + [16, 6, 2],
                pool_every=0, v_dve_edge=2, bufs_z=2, bufs_v=2, bufs_t=2,
                t_act=2)


def _get_nc(**cfg):
    cfg = {**BEST_CFG, **cfg}
    key = tuple(sorted(
        (k, tuple(v) if isinstance(v, list) else v) for k, v in cfg.items()))
    if key not in _CACHE:
        _CACHE[key] = _build_bass(**cfg)
    return _CACHE[key]


def run_sharded(x, w, cfg=None, **run_kwargs):
    import ml_dtypes
    from concourse.bass_utils import run_bass_kernel_spmd

    scale = _taps_from_w(w)
    nc = _get_nc(**(cfg or {}))

    x = np.asarray(x, dtype=np.float32)
    in_maps = []
    for k in range(NCORES):
        xk = np.zeros((NIMG, H, SW), ml_dtypes.bfloat16)
        xk[:, :, 2:2 + W] = (
            x[:, k * CPC:(k + 1) * CPC].reshape(NIMG, H, W)
            * np.float32(scale)).astype(ml_dtypes.bfloat16)
        in_maps.append({"x": xk})

    res = run_bass_kernel_spmd(nc, in_maps, core_ids=list(range(NCORES)),
                               **run_kwargs)

    full = np.empty((N, C, OH, OW), dtype=np.float32)
    for k in range(NCORES):
        e = res.results[k]["out_e"].reshape(N, CPC, OH, PW)
        o = res.results[k]["out_o"].reshape(N, CPC, OH, PW)
        # interleave column planes: out[..., 2j] = e[..., j], 2j+1 = o[..., j]
        full[:, k * CPC:(k + 1) * CPC] = np.stack(
            [e, o], axis=-1).reshape(N, CPC, OH, 2 * PW)[..., :OW].astype(
            np.float32)
    return full, res


def kernel(x, w):
    full, _ = run_sharded(x, w)
    return full


# revision 21
# speedup vs baseline: 1.0891x; 1.0091x over previous
"""Stride-2 bilinear upsampling (block-diagonal conv_transpose2d) on 8 NeuronCores.

v4 design: bf16 end-to-end on device, and fully COLUMN-PLANAR on device.

The device never materializes interleaved output columns: it computes the
even-column plane and odd-column plane separately and writes two bf16 HBM
tensors; the host interleaves them (pure layout, part of the unshard) and
upcasts to f32.  The a^2 = 1/16 filter scale is folded into the host-side
bf16 cast (power of two -> lossless).  This removes every strided-dst DVE
op, so all tensor_tensor ops hit the 2x bf16 uop (~0.59 ns/elem measured)
and all tensor_scalar ops can be flat-contiguous for the 4x uop
(~0.28 ns/elem; 2D access patterns demote ts to 1x, measured).

Math (per 1D axis, taps f = [1,3,3,1]*a): with S = a^2*X (host), ghosts 0:
  W-pass (cols, planar):  Ue[u,j] = 3S[j+1] + S[j+2] = T[j+1] + S[j+2]
                          Uo[u,j] = S[j+1] + 3S[j+2] = S[j+1] + T[j+2]
                          (T = 3S via flat ts 4x)
  H-pass (rows, per plane P in {e,o}):  Zp[2m]   = 3Up[m] + Up[m+1]
                                        Zp[2m+1] = Up[m] + 3Up[m+1]
                          (Vp = 3Up on ACT; row-interleaved dst is packed
                           in the inner dim so tt stays 2x)
Engine split: DVE = T + W + most H; ACT = V (+T on non-edge strips);
Pool (GpSimd tt, slow ~3.8 ns/elem) absorbs the odd plane of every k-th
strip as a self-contained chain (tt, tt, own SWDGE DMA).

Channel-parallel: 32 ch x 4 batch = 128 images/core, one per partition.
"""

import numpy as np

N, C, H, W = 4, 256, 128, 128
OH, OW = 258, 258
PW = OW // 2 + 1           # plane width padded to 130 (even => 2x uop, 4B-aligned rows)
NCORES = 8
CPC = C // NCORES          # 32 channels per core
NIMG = N * CPC             # 128 images per core (one per SBUF partition)
SW = W + 4                 # S width: [pad][ghostL][128 data][ghostR][pad]

_CACHE = {}


def _legalize_waits(nc, mybir):
    """Split multi-wait sync_info into standalone single-wait EventSemaphore
    instructions (this build encodes at most one sync-wait per instruction)."""
    n = 0
    for func in nc.m.functions:
        for block in func.blocks:
            out = []
            for inst in block.instructions:
                si = inst.sync_info
                if si is not None and si.on_wait is not None and len(si.on_wait) > 1:
                    waits = list(si.on_wait)
                    for k, w in enumerate(waits[:-1]):
                        out.append(mybir.InstEventSemaphore(
                            name=f"{inst.name}-hw{k}",
                            opcode="EventSemaphore",
                            engine=inst.engine,
                            ins=[], outs=[],
                            sync_info=mybir.SyncInfo(on_wait=[w], on_update=[]),
                        ))
                        n += 1
                    inst.sync_info = mybir.SyncInfo(
                        on_wait=[waits[-1]], on_update=list(si.on_update))
                out.append(inst)
            block.instructions = out
    return n


def _build_bass(strips=None, bufs_v=3, bufs_z=4, bufs_t=3,
                pool_every=3, t_act=1, v_act=1, v_dve_edge=4,
                out_scalar_every=0, out_delay=2):
    """Per-core view: x[128,128,128]bf16 (pre-scaled by a^2) ->
    out_e/out_o[128,258,129]bf16 (even/odd column planes).

    pool_every: every k-th non-edge strip's odd plane H-pass runs on
    GpSimd (0 = never).  t_act/v_act: put T=3S / V=3U on ACT for non-edge
    strips (else DVE flat ts 4x).  v_dve_edge: first/last k strips keep
    everything on DVE for short fill/drain latency.
    """
    import concourse.bass as bass
    import concourse.mybir as mybir
    from concourse.tile import TileContext

    bf16 = mybir.dt.bfloat16
    Copy = mybir.ActivationFunctionType.Copy
    add = mybir.AluOpType.add
    if strips is None:
        strips = [1, 1, 2, 4] + [8] * 14 + [4, 2, 1, 1]
    assert sum(strips) == H
    nstrips = len(strips)
    hs_max = max(strips)
    m0s = list(np.cumsum([0] + strips[:-1]))

    nc = bass.Bass()
    x = nc.dram_tensor("x", [NIMG, H, SW], bf16, kind="ExternalInput")
    out_e = nc.dram_tensor("out_e", [NIMG, OH, PW], bf16,
                           kind="ExternalOutput")
    out_o = nc.dram_tensor("out_o", [NIMG, OH, PW], bf16,
                           kind="ExternalOutput")

    with TileContext(nc) as tc:
        with tc.tile_pool(name="p", bufs=2) as pool:
            # persistent S: input rows land at cols 2..129; ghost zeros at
            # cols 1/130; pad cols 0/131 also zeroed (T reads full rows so
            # its flat access pattern stays contiguous).
            S = pool.tile([NIMG, H, SW], bf16, tag="S", bufs=1)

            # persistent planar U [130 x 129] per parity; ghost rows 0/129.
            Ue = pool.tile([NIMG, H + 2, PW], bf16, tag="Ue", bufs=1)
            Uo = pool.tile([NIMG, H + 2, PW], bf16, tag="Uo", bufs=1)
            for Up in (Ue, Uo):
                nc.gpsimd.memset(Up[:, 0:1, :], 0.0)
                nc.gpsimd.memset(Up[:, H + 1:H + 2, :], 0.0)

            # input chunks (host-padded to S layout: fully contiguous
            # 6KB-per-partition lines) aligned to strip boundaries
            in_chunks = [(0, 8), (8, 24), (32, 24),
                         (56, 24), (80, 24), (104, 24)]
            assert sum(n for _, n in in_chunks) == H
            for r0, nr in in_chunks:
                nc.sync.dma_start(out=S[:, r0:r0 + nr, :],
                                  in_=x[:, r0:r0 + nr, :])

            pending = []   # (due_strip, zt_view, dram, row0, row1)

            def flush_pending(now):
                while pending and pending[0][0] <= now:
                    _, ztv, dram, r0_, r1_ = pending.pop(0)
                    nc.scalar.dma_start(out=dram[:, r0_:r1_, :], in_=ztv)

            npool = 0
            for s in range(nstrips):
                hs = strips[s]
                m0 = int(m0s[s])
                n_m = hs + (1 if s == nstrips - 1 else 0)
                edge = s < v_dve_edge or s >= nstrips - v_dve_edge

                # T = 3S over full S rows (flat on DVE for 4x; ACT is
                # stride-blind). T local col c corresponds to S col c.
                tt_ = pool.tile([NIMG, hs_max, SW], bf16, tag="tt",
                                bufs=bufs_t)
                on_act = t_act and not edge and (t_act == 1 or s % 2 == 0)
                if on_act:
                    nc.scalar.activation(tt_[:, 0:hs, :], S[:, m0:m0 + hs, :],
                                         Copy, scale=3.0)
                else:
                    nc.vector.tensor_scalar_mul(
                        tt_[:, 0:hs, :], S[:, m0:m0 + hs, :], 3.0)

                # W-pass (planar, packed tt 2x) -> U rows m0+1 .. m0+hs
                nc.vector.tensor_tensor(
                    out=Ue[:, m0 + 1:m0 + hs + 1, :],
                    in0=tt_[:, 0:hs, 1:W + 3],
                    in1=S[:, m0:m0 + hs, 2:W + 4], op=add)
                nc.vector.tensor_tensor(
                    out=Uo[:, m0 + 1:m0 + hs + 1, :],
                    in0=S[:, m0:m0 + hs, 1:W + 3],
                    in1=tt_[:, 0:hs, 2:W + 4], op=add)

                # whole odd plane of every k-th non-edge strip -> Pool
                pool_o = (pool_every and not edge
                          and (s - v_dve_edge) % pool_every == pool_every - 1)
                if pool_o:
                    npool += 1

                for par, Up, dram in ((0, Ue, out_e), (1, Uo, out_o)):
                    on_pool = pool_o and par == 1
                    # V = 3U rows m0 .. m0+n_m (flat ts 4x on DVE, or ACT)
                    vt_full = pool.tile([NIMG, hs_max + 2, PW], bf16,
                                        tag=f"vt{par}", bufs=bufs_v)
                    vt = vt_full[:, 0:n_m + 1, :]
                    if v_act and not edge:
                        nc.scalar.activation(vt, U_rows(Up, m0, n_m),
                                             Copy, scale=3.0)
                    else:
                        nc.vector.tensor_scalar_mul(
                            vt, U_rows(Up, m0, n_m), 3.0)

                    # H-pass: Zp rows 2m0 .. 2(m0+n_m)-1
                    he = nc.gpsimd if on_pool else nc.vector
                    zt_full = pool.tile([NIMG, 2 * hs_max + 2, PW], bf16,
                                        tag=f"zt{par}", bufs=bufs_z)
                    zt = zt_full[:, 0:2 * n_m, :]
                    he.tensor_tensor(
                        out=zt[:, 0:2 * n_m:2, :],
                        in0=vt[:, 0:n_m, :],
                        in1=Up[:, m0 + 1:m0 + n_m + 1, :], op=add)
                    he.tensor_tensor(
                        out=zt[:, 1:2 * n_m:2, :],
                        in0=Up[:, m0:m0 + n_m, :],
                        in1=vt[:, 1:n_m + 1, :], op=add)

                    r0_, r1_ = 2 * m0, 2 * (m0 + n_m)
                    if out_scalar_every and s % out_scalar_every == 1:
                        pending.append((s + out_delay, zt[:, :, :],
                                        dram, r0_, r1_))
                    else:
                        oeng = nc.gpsimd if par == 1 else nc.sync
                        oeng.dma_start(out=dram[:, r0_:r1_, :],
                                       in_=zt[:, :, :])
                flush_pending(s)
            flush_pending(nstrips)

    _legalize_waits(nc, mybir)
    return nc


def U_rows(Up, m0, n_m):
    return Up[:, m0:m0 + n_m + 1, :]


def _taps_from_w(w):
    """Recover separable 4-tap filter f (filt = outer(f, f)) from w[0, 0];
    return a^2 where f = [a, 3a, 3a, a]."""
    filt = np.asarray(w, dtype=np.float32)[0, 0]
    j = int(np.argmax(np.abs(np.diag(filt))))
    f = filt[:, j] / np.float32(np.sqrt(filt[j, j]))
    assert np.allclose(np.outer(f, f), filt, atol=1e-5), "filter not separable"
    assert abs(f[0] - f[3]) < 1e-6 and abs(f[1] - f[2]) < 1e-6, "not symmetric"
    assert abs(f[1] - 3 * f[0]) < 1e-5, "not the 3:1 bilinear tap"
    return float(f[0]) * float(f[0])


BEST_CFG = dict(strips=[8] + [24] * 4 + [16, 8],
                pool_every=0, v_dve_edge=1, bufs_z=2, bufs_v=2, bufs_t=2,
                t_act=2)


def _get_nc(**cfg):
    cfg = {**BEST_CFG, **cfg}
    key = tuple(sorted(
        (k, tuple(v) if isinstance(v, list) else v) for k, v in cfg.items()))
    if key not in _CACHE:
        _CACHE[key] = _build_bass(**cfg)
    return _CACHE[key]


def run_sharded(x, w, cfg=None, **run_kwargs):
    import ml_dtypes
    from concourse.bass_utils import run_bass_kernel_spmd

    scale = _taps_from_w(w)
    nc = _get_nc(**(cfg or {}))

    x = np.asarray(x, dtype=np.float32)
    in_maps = []
    for k in range(NCORES):
        xk = np.zeros((NIMG, H, SW), ml_dtypes.bfloat16)
        xk[:, :, 2:2 + W] = (
            x[:, k * CPC:(k + 1) * CPC].reshape(NIMG, H, W)
            * np.float32(scale)).astype(ml_dtypes.bfloat16)
        in_maps.append({"x": xk})

    res = run_bass_kernel_spmd(nc, in_maps, core_ids=list(range(NCORES)),
                               **run_kwargs)

    full = np.empty((N, C, OH, OW), dtype=np.float32)
    for k in range(NCORES):
        e = res.results[k]["out_e"].reshape(N, CPC, OH, PW)
        o = res.results[k]["out_o"].reshape(N, CPC, OH, PW)
        # interleave column planes: out[..., 2j] = e[..., j], 2j+1 = o[..., j]
        full[:, k * CPC:(k + 1) * CPC] = np.stack(
            [e, o], axis=-1).reshape(N, CPC, OH, 2 * PW)[..., :OW].astype(
            np.float32)
    return full, res


def kernel(x, w):
    full, _ = run_sharded(x, w)
    return full
